# revision 11
# baseline (speedup 1.0000x reference)
"""DiceLoss kernel for 8 Trainium2 NeuronCores.

Reference computation:
    inter[b,c] = sum_p pred[b,c,p] * target[b,c,p]          # [4, 8]
    denom      = sum(pred) + sum(target) + 1.0              # scalar
    loss_bc    = 2 * (inter + 1) / denom
    total      = sum_b( sum_c(loss_bc[b]) * 8**(b-4) ) / 4
    out        = 1 - total

Sharding: flatten (b, c) -> 32 rows of 2M pixels; core k takes rows
4k..4k+3.  Each core views its 8 Mi-pixel slab as 128 partitions and
streams 64 MiB (pred+target) through a raw-bass double-buffered
pipeline.

Per-engine load shaping: the HWDGE splits an n-row descriptor into
equal integer blocks over <=16 SDMA engines, always starting at engine
0 ([0:128] -> 16x8, [0:120] -> 15x8 with engine 15 idle).  Traces show
engine 15 (or 0) of a core's bank intermittently running at ~0.8x,
which stalls the whole stream.  Engine 0 cannot be underloaded (it
always receives the first block), but engine 15 can: each piece issues
an A-descriptor [0:128] x w_A plus a B-descriptor [0:120] x w_B, so
partitions 120-127 ("short", engine 15) carry T_S = 49936 elems/row
while partitions 0-119 ("long") carry T_L = 66576 (ratio 0.75).  The
partition->(b,c)-group map is redefined to keep 2 short partitions in
every group (gmat encodes it), so per-group pixel conservation holds.

Engines: DVE computes per-piece dot partials (A and B ranges) plus
sum(pred+target) over the B columns via add/add scalar_tensor_tensor
(only the combined sum is needed); ACT computes per-partition sums
over the wider A columns (pred, target).  This keeps ACT (~110us) and
DVE (~130us) both clearly under the DMA stream time.  A PE matmul
against a [128, 8] mask matrix folds partitions; an [8, 2] result per
core is combined on the host.  DRAM tensors carry 16 never-read guard
rows on each side.
"""

from contextlib import ExitStack

import numpy as np

N, C, P = 4, 8, 2097152
NCORES = 8
ROWS = N * C                      # 32 (b,c) rows
RPC = ROWS // NCORES              # 4 rows per core
FREE = RPC * P // 128             # 65536 elems/partition if uniform
GUARD = 16                        # unread guard rows flanking each tensor

T_L = 66576                       # long-partition row length (0-119)
T_S = 49936                       # short-partition row length (120-127)
LONGP = 120
WA = [3121] * 14 + [2048, 2048, 1536, 610]   # A-piece widths (sum T_S)
WB = [1040] * 14 + [683, 683, 512, 202]      # B-piece widths (sum T_L-T_S)
NP = len(WA)                      # 18 pieces
BUFS = 6
OA = [sum(WA[:i]) for i in range(NP)]
OB = [T_S + sum(WB[:i]) for i in range(NP)]

# cols layout: dA [0:NP], dB [NP:2NP], pA [2NP:3NP], tA [3NP:4NP],
# sB [4NP:5NP], fin [5NP:5NP+2]
CDA, CDB, CPA, CTA, CSB, CFIN = 0, NP, 2 * NP, 3 * NP, 4 * NP, 5 * NP
NCOLS = 5 * NP + 2

_CACHE = {}


def _build_bass():
    import concourse.bass as bass
    import concourse.mybir as mybir

    f32 = mybir.dt.float32
    nc = bass.Bass("TRN2", target_bir_lowering=False, debug=False,
                   num_devices=NCORES)

    pred = nc.dram_tensor("pred", [128 + 2 * GUARD, T_L], f32,
                          kind="ExternalInput").ap()
    targ = nc.dram_tensor("target", [128 + 2 * GUARD, T_L], f32,
                          kind="ExternalInput").ap()
    gmat = nc.dram_tensor("gmat", [128, 8], f32, kind="ExternalInput").ap()
    out = nc.dram_tensor("out", [8, 2], f32, kind="ExternalOutput").ap()

    predf = pred[GUARD:GUARD + 128, :]
    targf = targ[GUARD:GUARD + 128, :]

    AX = mybir.AxisListType.X
    MUL = mybir.AluOpType.mult
    ADD = mybir.AluOpType.add
    COPY = mybir.ActivationFunctionType.Copy

    with ExitStack() as ctx:
        e = ctx.enter_context
        slotw = max(a + b for a, b in zip(WA, WB))
        pred_sl = [e(nc.sbuf_tensor(f"pred_sl{i}", [128, slotw], f32))
                   for i in range(BUFS)]
        targ_sl = [e(nc.sbuf_tensor(f"targ_sl{i}", [128, slotw], f32))
                   for i in range(BUFS)]
        cols = e(nc.sbuf_tensor([128, NCOLS], f32))
        dummy_v = e(nc.sbuf_tensor([128, 2 * NP], f32))
        dummy_a = e(nc.sbuf_tensor([128, 2 * NP], f32))
        dummy_q = e(nc.sbuf_tensor([128, NP], f32))
        g_sb = e(nc.sbuf_tensor([128, 8], f32))
        osb = e(nc.sbuf_tensor([8, 2], f32))
        ps = e(nc.psum_tensor([8, 2], f32))

        # per-slot DMA-completion sems; pred/targ A+B of one piece share
        # a sem (4 descs x 16 = 64); waiters only test full-piece values.
        ss = [e(nc.semaphore(f"ss{i}")) for i in range(BUFS)]
        sv = e(nc.semaphore())   # DVE progress (memset + 1/piece + fold)
        sa = e(nc.semaphore())   # ACT progress (2/piece)
        so = e(nc.semaphore())   # output ready
        sg = e(nc.semaphore())   # gmat loaded
        spe = e(nc.semaphore())  # PE matmul done

        block = e(nc.Block(no_gpsimd_drain=True))

        @block.sync
        def _(sync):
            for i in range(NP):
                if i >= BUFS:
                    done = i - BUFS + 1
                    sync.wait_ge(sv, 1 + done)
                    sync.wait_ge(sa, 2 * done)
                s = i % BUFS
                wa, wb = WA[i], WB[i]
                sync.dma_start(
                    pred_sl[s][:, 0:wa], predf[:, OA[i]:OA[i] + wa]
                ).then_inc(ss[s], 16)
                sync.dma_start(
                    pred_sl[s][0:LONGP, wa:wa + wb],
                    predf[0:LONGP, OB[i]:OB[i] + wb]
                ).then_inc(ss[s], 16)
                sync.dma_start(
                    targ_sl[s][:, 0:wa], targf[:, OA[i]:OA[i] + wa]
                ).then_inc(ss[s], 16)
                sync.dma_start(
                    targ_sl[s][0:LONGP, wa:wa + wb],
                    targf[0:LONGP, OB[i]:OB[i] + wb]
                ).then_inc(ss[s], 16)
            sync.wait_ge(so, 1)
            sync.dma_start(out, osb[:]).then_inc(sg, 16)

        @block.vector
        def _(vector):
            nc.vector.memset(cols[:], 0.0).then_inc(sv, 1)
            for i in range(NP):
                s = i % BUFS
                wa, wb = WA[i], WB[i]
                vector.wait_ge(ss[s], 64 * (i // BUFS + 1))
                nc.vector.scalar_tensor_tensor(
                    out=dummy_v[:, 2 * i:2 * i + 1].broadcast_to((128, wa)),
                    in0=pred_sl[s][:, 0:wa],
                    scalar=1.0,
                    in1=targ_sl[s][:, 0:wa],
                    op0=MUL,
                    op1=MUL,
                    accum_out=cols[:, CDA + i:CDA + i + 1],
                )
                nc.vector.scalar_tensor_tensor(
                    out=dummy_v[0:LONGP, 2 * i + 1:2 * i + 2]
                        .broadcast_to((LONGP, wb)),
                    in0=pred_sl[s][0:LONGP, wa:wa + wb],
                    scalar=1.0,
                    in1=targ_sl[s][0:LONGP, wa:wa + wb],
                    op0=MUL,
                    op1=MUL,
                    accum_out=cols[0:LONGP, CDB + i:CDB + i + 1],
                )
                # sum(pred + target) over the B columns in one pass
                nc.vector.scalar_tensor_tensor(
                    out=dummy_q[0:LONGP, i:i + 1].broadcast_to((LONGP, wb)),
                    in0=pred_sl[s][0:LONGP, wa:wa + wb],
                    scalar=0.0,
                    in1=targ_sl[s][0:LONGP, wa:wa + wb],
                    op0=ADD,
                    op1=ADD,
                    accum_out=cols[0:LONGP, CSB + i:CSB + i + 1],
                ).then_inc(sv, 1)
            # fold piece columns
            nc.vector.reduce_sum(cols[:, CFIN:CFIN + 1],
                                 cols[:, CDA:CDA + 2 * NP], axis=AX)
            vector.wait_ge(sa, 2 * NP)
            nc.vector.reduce_sum(cols[:, CFIN + 1:CFIN + 2],
                                 cols[:, CPA:CPA + 3 * NP],
                                 axis=AX).then_inc(sv, 1)
            vector.wait_ge(spe, 1)
            nc.vector.tensor_copy(osb[:], ps[:]).then_inc(so, 1)

        @block.scalar
        def _(scalar):
            # gmat load rides the idle ACT HWDGE ring
            scalar.dma_start(g_sb[:], gmat).then_inc(sg, 16)
            scalar.wait_ge(sv, 1)
            for i in range(NP):
                s = i % BUFS
                wa = WA[i]
                scalar.wait_ge(ss[s], 64 * (i // BUFS + 1))
                nc.scalar.activation(
                    dummy_a[:, 2 * i:2 * i + 1].broadcast_to((128, wa)),
                    pred_sl[s][:, 0:wa], COPY,
                    accum_out=cols[:, CPA + i:CPA + i + 1],
                ).then_inc(sa, 1)
                nc.scalar.activation(
                    dummy_a[:, 2 * i + 1:2 * i + 2].broadcast_to((128, wa)),
                    targ_sl[s][:, 0:wa], COPY,
                    accum_out=cols[:, CTA + i:CTA + i + 1],
                ).then_inc(sa, 1)

        @block.tensor
        def _(tensor):
            tensor.wait_ge(sg, 16)
            tensor.wait_ge(sv, NP + 2)
            nc.tensor.matmul(ps[:], g_sb[:], cols[:, CFIN:CFIN + 2],
                             start=True, stop=True).then_inc(spe, 1)

    return nc


def _gmat() -> np.ndarray:
    g = np.zeros((128, 8), dtype=np.float32)
    for b in range(RPC):
        g[30 * b:30 * b + 30, b] = 1.0          # long partitions of group b
        g[120 + 2 * b:122 + 2 * b, b] = 1.0     # short partitions of group b
    g[:, 4] = 1.0                               # col 4: all-ones (global sum)
    return g


def _pack(core_rows: np.ndarray) -> np.ndarray:
    """[RPC, P] rows -> guarded [128+2G, T_L] slab (shorts padded)."""
    dst = np.empty((128 + 2 * GUARD, T_L), dtype=np.float32)
    for g in range(RPC):
        row = core_rows[g]
        dst[GUARD + 30 * g:GUARD + 30 * g + 30] = (
            row[:30 * T_L].reshape(30, T_L)
        )
        dst[GUARD + 120 + 2 * g:GUARD + 122 + 2 * g, :T_S] = (
            row[30 * T_L:].reshape(2, T_S)
        )
    return dst


def _make_in_maps(pred: np.ndarray, target: np.ndarray):
    predr = np.ascontiguousarray(pred, dtype=np.float32).reshape(ROWS, P)
    targr = np.ascontiguousarray(target, dtype=np.float32).reshape(ROWS, P)
    g = _gmat()
    maps = []
    for k in range(NCORES):
        maps.append({
            "pred": _pack(predr[k * RPC:(k + 1) * RPC]),
            "target": _pack(targr[k * RPC:(k + 1) * RPC]),
            "gmat": g,
        })
    return maps


def _run(pred: np.ndarray, target: np.ndarray, trace: bool = False):
    from concourse.bass_utils import run_bass_kernel_spmd

    if "nc" not in _CACHE:
        _CACHE["nc"] = _build_bass()
    nc = _CACHE["nc"]
    in_maps = _make_in_maps(pred, target)
    return run_bass_kernel_spmd(nc, in_maps, core_ids=list(range(NCORES)),
                                trace=trace)


def _combine(results) -> np.ndarray:
    inter = np.empty(ROWS, dtype=np.float64)
    sums = 0.0
    for k in range(NCORES):
        o = np.asarray(results[k]["out"], dtype=np.float64)   # [8, 2]
        inter[k * RPC:(k + 1) * RPC] = o[0:RPC, 0]
        sums += o[4, 1]
    denom = sums + 1.0
    loss_bc = 2.0 * (inter.reshape(N, C) + 1.0) / denom
    weights = np.float64(C) ** (np.arange(N, dtype=np.float64) - N)
    total = (loss_bc.sum(axis=1) * weights).sum() / N
    return np.array(1.0 - total, dtype=np.float32)


def kernel(pred: np.ndarray, target: np.ndarray) -> np.ndarray:
    pred = np.asarray(pred, dtype=np.float32)
    target = np.asarray(target, dtype=np.float32)
    res = _run(pred, target, trace=False)
    return _combine(res.results)


# revision 13
# speedup vs baseline: 1.0085x; 1.0085x over previous
"""DiceLoss kernel for 8 Trainium2 NeuronCores.

Reference computation:
    inter[b,c] = sum_p pred[b,c,p] * target[b,c,p]          # [4, 8]
    denom      = sum(pred) + sum(target) + 1.0              # scalar
    loss_bc    = 2 * (inter + 1) / denom
    total      = sum_b( sum_c(loss_bc[b]) * 8**(b-4) ) / 4
    out        = 1 - total

Sharding: flatten (b, c) -> 32 rows of 2M pixels; core k takes rows
4k..4k+3.  Each core views its 8 Mi-pixel slab as 128 partitions and
streams 64 MiB (pred+target) through a raw-bass double-buffered
pipeline.

Per-engine load shaping: the HWDGE splits an n-row descriptor into
equal integer blocks over <=16 SDMA engines, always starting at engine
0 ([0:128] -> 16x8, [0:120] -> 15x8 with engine 15 idle).  Traces show
engine 15 (or 0) of a core's bank intermittently running at ~0.8x,
which stalls the whole stream.  Engine 0 cannot be underloaded (it
always receives the first block), but engine 15 can: each piece issues
an A-descriptor [0:128] x w_A plus a B-descriptor [0:120] x w_B, so
partitions 120-127 ("short", engine 15) carry T_S = 49936 elems/row
while partitions 0-119 ("long") carry T_L = 66576 (ratio 0.75).  The
partition->(b,c)-group map is redefined to keep 2 short partitions in
every group (gmat encodes it), so per-group pixel conservation holds.

Engines: DVE computes per-piece dot partials (A and B ranges) plus
sum(pred+target) over the B columns via add/add scalar_tensor_tensor
(only the combined sum is needed); ACT computes per-partition sums
over the wider A columns (pred, target).  This keeps ACT (~110us) and
DVE (~130us) both clearly under the DMA stream time.  A PE matmul
against a [128, 8] mask matrix folds partitions; an [8, 2] result per
core is combined on the host.  DRAM tensors carry 16 never-read guard
rows on each side.
"""

from contextlib import ExitStack

import numpy as np

N, C, P = 4, 8, 2097152
NCORES = 8
ROWS = N * C                      # 32 (b,c) rows
RPC = ROWS // NCORES              # 4 rows per core
FREE = RPC * P // 128             # 65536 elems/partition if uniform
GUARD = 16                        # unread guard rows flanking each tensor

T_L = 66576                       # long-partition row length (0-119)
T_S = 49936                       # short-partition row length (120-127)
LONGP = 120
WA = [4160] * 11 + [2048, 1424, 704]   # A-piece widths (sum = T_S)
NPA = len(WA)                     # 14 A-pieces
OA = [sum(WA[:i]) for i in range(NPA)]
WBP = 4160                        # B-piece width (4 x 4160 = T_L - T_S)
BP = [2, 5, 8, 11]                # global pieces that also carry a B-piece
NPB = len(BP)
BUFS = 4                          # A-slot ring depth
BUFSB = 2                         # B-slot ring depth

# cols layout: dA, dB, pA, tA, sB, fin
CDA, CDB, CPA, CTA = 0, NPA, NPA + NPB, 2 * NPA + NPB
CSB, CFIN = 3 * NPA + NPB, 3 * NPA + 2 * NPB
NCOLS = 3 * NPA + 2 * NPB + 2

_CACHE = {}


def _build_bass():
    import concourse.bass as bass
    import concourse.mybir as mybir

    f32 = mybir.dt.float32
    nc = bass.Bass("TRN2", target_bir_lowering=False, debug=False,
                   num_devices=NCORES)

    pred = nc.dram_tensor("pred", [128 + 2 * GUARD, T_L], f32,
                          kind="ExternalInput").ap()
    targ = nc.dram_tensor("target", [128 + 2 * GUARD, T_L], f32,
                          kind="ExternalInput").ap()
    gmat = nc.dram_tensor("gmat", [128, 8], f32, kind="ExternalInput").ap()
    out = nc.dram_tensor("out", [8, 2], f32, kind="ExternalOutput").ap()

    predf = pred[GUARD:GUARD + 128, :]
    targf = targ[GUARD:GUARD + 128, :]

    AX = mybir.AxisListType.X
    MUL = mybir.AluOpType.mult
    ADD = mybir.AluOpType.add
    COPY = mybir.ActivationFunctionType.Copy

    with ExitStack() as ctx:
        e = ctx.enter_context
        slotw = max(WA)
        pred_sl = [e(nc.sbuf_tensor(f"pred_sl{i}", [128, slotw], f32))
                   for i in range(BUFS)]
        targ_sl = [e(nc.sbuf_tensor(f"targ_sl{i}", [128, slotw], f32))
                   for i in range(BUFS)]
        pred_bl = [e(nc.sbuf_tensor(f"pred_bl{i}", [128, WBP], f32))
                   for i in range(BUFSB)]
        targ_bl = [e(nc.sbuf_tensor(f"targ_bl{i}", [128, WBP], f32))
                   for i in range(BUFSB)]
        cols = e(nc.sbuf_tensor([128, NCOLS], f32))
        dummy_v = e(nc.sbuf_tensor([128, NPA + 2 * NPB], f32))
        dummy_a = e(nc.sbuf_tensor([128, 2 * NPA], f32))
        g_sb = e(nc.sbuf_tensor([128, 8], f32))
        osb = e(nc.sbuf_tensor([8, 2], f32))
        ps = e(nc.psum_tensor([8, 2], f32))

        # per-slot DMA-completion sems; pred+targ of one piece share a
        # sem (2 descs x 16 = 32); waiters only test full-piece values.
        ss = [e(nc.semaphore(f"ss{i}")) for i in range(BUFS)]
        ssb = [e(nc.semaphore(f"ssb{i}")) for i in range(BUFSB)]
        sv = e(nc.semaphore())   # DVE progress (memset + 1/A-piece + fold)
        sa = e(nc.semaphore())   # ACT progress (2/A-piece)
        svb = e(nc.semaphore())  # DVE B progress (1/B-piece)
        so = e(nc.semaphore())   # output ready
        sg = e(nc.semaphore())   # gmat loaded
        spe = e(nc.semaphore())  # PE matmul done

        block = e(nc.Block(no_gpsimd_drain=True))

        @block.sync
        def _(sync):
            for i in range(NPA):
                if i >= BUFS:
                    done = i - BUFS + 1
                    sync.wait_ge(sv, 1 + done)
                    sync.wait_ge(sa, 2 * done)
                s = i % BUFS
                wa = WA[i]
                sync.dma_start(
                    pred_sl[s][:, 0:wa], predf[:, OA[i]:OA[i] + wa]
                ).then_inc(ss[s], 16)
                sync.dma_start(
                    targ_sl[s][:, 0:wa], targf[:, OA[i]:OA[i] + wa]
                ).then_inc(ss[s], 16)
                if i in BP:
                    j = BP.index(i)
                    if j >= BUFSB:
                        sync.wait_ge(svb, j - BUFSB + 1)
                    sb = j % BUFSB
                    off = T_S + j * WBP
                    sync.dma_start(
                        pred_bl[sb][0:LONGP, :],
                        predf[0:LONGP, off:off + WBP]
                    ).then_inc(ssb[sb], 16)
                    sync.dma_start(
                        targ_bl[sb][0:LONGP, :],
                        targf[0:LONGP, off:off + WBP]
                    ).then_inc(ssb[sb], 16)
            sync.wait_ge(so, 1)
            sync.dma_start(out, osb[:]).then_inc(sg, 16)

        @block.vector
        def _(vector):
            nc.vector.memset(cols[:], 0.0).then_inc(sv, 1)
            for i in range(NPA):
                s = i % BUFS
                wa = WA[i]
                vector.wait_ge(ss[s], 32 * (i // BUFS + 1))
                nc.vector.scalar_tensor_tensor(
                    out=dummy_v[:, i:i + 1].broadcast_to((128, wa)),
                    in0=pred_sl[s][:, 0:wa],
                    scalar=1.0,
                    in1=targ_sl[s][:, 0:wa],
                    op0=MUL,
                    op1=MUL,
                    accum_out=cols[:, CDA + i:CDA + i + 1],
                ).then_inc(sv, 1)
                if i in BP:
                    j = BP.index(i)
                    sb = j % BUFSB
                    vector.wait_ge(ssb[sb], 32 * (j // BUFSB + 1))
                    nc.vector.scalar_tensor_tensor(
                        out=dummy_v[0:LONGP, NPA + 2 * j:NPA + 2 * j + 1]
                            .broadcast_to((LONGP, WBP)),
                        in0=pred_bl[sb][0:LONGP, :],
                        scalar=1.0,
                        in1=targ_bl[sb][0:LONGP, :],
                        op0=MUL,
                        op1=MUL,
                        accum_out=cols[0:LONGP, CDB + j:CDB + j + 1],
                    )
                    # sum(pred + target) over the B piece in one pass
                    nc.vector.scalar_tensor_tensor(
                        out=dummy_v[0:LONGP, NPA + 2 * j + 1:NPA + 2 * j + 2]
                            .broadcast_to((LONGP, WBP)),
                        in0=pred_bl[sb][0:LONGP, :],
                        scalar=0.0,
                        in1=targ_bl[sb][0:LONGP, :],
                        op0=ADD,
                        op1=ADD,
                        accum_out=cols[0:LONGP, CSB + j:CSB + j + 1],
                    ).then_inc(svb, 1)
            # fold piece columns
            nc.vector.reduce_sum(cols[:, CFIN:CFIN + 1],
                                 cols[:, CDA:CDA + NPA + NPB], axis=AX)
            vector.wait_ge(sa, 2 * NPA)
            nc.vector.reduce_sum(cols[:, CFIN + 1:CFIN + 2],
                                 cols[:, CPA:CPA + 2 * NPA + NPB],
                                 axis=AX).then_inc(sv, 1)
            vector.wait_ge(spe, 1)
            nc.vector.tensor_copy(osb[:], ps[:]).then_inc(so, 1)

        @block.scalar
        def _(scalar):
            # gmat load rides the idle ACT HWDGE ring
            scalar.dma_start(g_sb[:], gmat).then_inc(sg, 16)
            scalar.wait_ge(sv, 1)
            for i in range(NPA):
                s = i % BUFS
                wa = WA[i]
                scalar.wait_ge(ss[s], 32 * (i // BUFS + 1))
                nc.scalar.activation(
                    dummy_a[:, 2 * i:2 * i + 1].broadcast_to((128, wa)),
                    pred_sl[s][:, 0:wa], COPY,
                    accum_out=cols[:, CPA + i:CPA + i + 1],
                ).then_inc(sa, 1)
                nc.scalar.activation(
                    dummy_a[:, 2 * i + 1:2 * i + 2].broadcast_to((128, wa)),
                    targ_sl[s][:, 0:wa], COPY,
                    accum_out=cols[:, CTA + i:CTA + i + 1],
                ).then_inc(sa, 1)

        @block.tensor
        def _(tensor):
            tensor.wait_ge(sg, 16)
            tensor.wait_ge(sv, NPA + 2)
            nc.tensor.matmul(ps[:], g_sb[:], cols[:, CFIN:CFIN + 2],
                             start=True, stop=True).then_inc(spe, 1)

    return nc


def _gmat() -> np.ndarray:
    g = np.zeros((128, 8), dtype=np.float32)
    for b in range(RPC):
        g[30 * b:30 * b + 30, b] = 1.0          # long partitions of group b
        g[120 + 2 * b:122 + 2 * b, b] = 1.0     # short partitions of group b
    g[:, 4] = 1.0                               # col 4: all-ones (global sum)
    return g


def _pack(core_rows: np.ndarray) -> np.ndarray:
    """[RPC, P] rows -> guarded [128+2G, T_L] slab (shorts padded)."""
    dst = np.empty((128 + 2 * GUARD, T_L), dtype=np.float32)
    for g in range(RPC):
        row = core_rows[g]
        dst[GUARD + 30 * g:GUARD + 30 * g + 30] = (
            row[:30 * T_L].reshape(30, T_L)
        )
        dst[GUARD + 120 + 2 * g:GUARD + 122 + 2 * g, :T_S] = (
            row[30 * T_L:].reshape(2, T_S)
        )
    return dst


def _make_in_maps(pred: np.ndarray, target: np.ndarray):
    predr = np.ascontiguousarray(pred, dtype=np.float32).reshape(ROWS, P)
    targr = np.ascontiguousarray(target, dtype=np.float32).reshape(ROWS, P)
    g = _gmat()
    maps = []
    for k in range(NCORES):
        maps.append({
            "pred": _pack(predr[k * RPC:(k + 1) * RPC]),
            "target": _pack(targr[k * RPC:(k + 1) * RPC]),
            "gmat": g,
        })
    return maps


def _run(pred: np.ndarray, target: np.ndarray, trace: bool = False):
    from concourse.bass_utils import run_bass_kernel_spmd

    if "nc" not in _CACHE:
        _CACHE["nc"] = _build_bass()
    nc = _CACHE["nc"]
    in_maps = _make_in_maps(pred, target)
    return run_bass_kernel_spmd(nc, in_maps, core_ids=list(range(NCORES)),
                                trace=trace)


def _combine(results) -> np.ndarray:
    inter = np.empty(ROWS, dtype=np.float64)
    sums = 0.0
    for k in range(NCORES):
        o = np.asarray(results[k]["out"], dtype=np.float64)   # [8, 2]
        inter[k * RPC:(k + 1) * RPC] = o[0:RPC, 0]
        sums += o[4, 1]
    denom = sums + 1.0
    loss_bc = 2.0 * (inter.reshape(N, C) + 1.0) / denom
    weights = np.float64(C) ** (np.arange(N, dtype=np.float64) - N)
    total = (loss_bc.sum(axis=1) * weights).sum() / N
    return np.array(1.0 - total, dtype=np.float32)


def kernel(pred: np.ndarray, target: np.ndarray) -> np.ndarray:
    pred = np.asarray(pred, dtype=np.float32)
    target = np.asarray(target, dtype=np.float32)
    res = _run(pred, target, trace=False)
    return _combine(res.results)
